# revision 1
# baseline (speedup 1.0000x reference)
"""Trainium2 Bass kernel for GNN edge-softmax attention message passing.

Strategy:
  - sort edges by destination row; 8 cores own contiguous 12544-row slices
  - 64-row blocks; per block, edges grouped by col-chunk (4 chunks of 25088
    cols so dma_gather's int16 indices reach the whole node table)
  - per 128-edge tile: gather [k|eigs|vhi|vlo|1] by col (1280B rows) and
    [q/sqrt(H)|eigs*exp(l0)] by local row (768B rows) with dma_gather,
    edge scores via DVE mul+reduce, exp on ACT, one-hot row matrices
    weighted by exp-scores on DVE, and a single bf16 matmul per tile
    accumulating [sum e0*v | d0] / [sum e1*v | d1] into PSUM
  - per-block raw [128,257] results go back to HBM; the final
    0.5*(P0/d0 + P1/d1) combine runs on host during unsharding
"""

import os
import sys
import types
import contextlib
import ctypes

import numpy as np

N = 100000
E = 3200000
H = 128
ED = 16
P6 = 6
NCORES = 8
R = 64                 # rows per block
CORE_ROWS = 12544      # 196 blocks of 64 rows
NPAD = CORE_ROWS * NCORES
NBLK = CORE_ROWS // R  # 196
NCHUNK = 4
CHUNK = NPAD // NCHUNK  # 25088
KVE_B = 1280           # bytes per kve row
QE_W = 192             # f32 per qe row (768B)
MAXCALL = 1024
LAST_EXEC_NS = None


def _install_axon_hooks():
    if "antenv.axon_hooks" in sys.modules:
        return
    mod = types.ModuleType("antenv.axon_hooks")
    _hook = [None]
    mod.set_axon_ntff_profile_hook = lambda h: _hook.__setitem__(0, h)
    mod.get_axon_ntff_profile_hook = lambda: _hook[0]
    sys.modules["antenv.axon_hooks"] = mod
    try:
        import antenv
        antenv.axon_hooks = mod
    except ImportError:
        pass
    try:
        from trn_agent_boot.trn_boot import _ntff_profile_via_ctypes
        h = _ntff_profile_via_ctypes("/opt/axon/libaxon_pjrt.so")
        if h is not None:
            mod.set_axon_ntff_profile_hook(h)
    except Exception:
        pass


def _prep(indices, path_type):
    """Sort/pad edges; returns per-core structures + the shared call plan."""
    row = indices[0].astype(np.int64)
    col = indices[1].astype(np.int64)
    core = row // CORE_ROWS
    blk = (row % CORE_ROWS) // R
    chunk = col // CHUNK
    # group key per edge: (core, blk, chunk)
    key = (core * NBLK + blk) * NCHUNK + chunk
    order = np.argsort(key, kind="stable")
    row_s, col_s, pt_s, key_s = row[order], col[order], path_type[order], key[order]
    ngroups = NCORES * NBLK * NCHUNK
    counts = np.bincount(key_s, minlength=ngroups).reshape(NCORES, NBLK, NCHUNK)
    # pad each (blk, chunk) group to a common multiple-of-128 size across cores
    gmax = counts.max(axis=0)  # [NBLK, NCHUNK]
    gpad = ((gmax + 127) // 128) * 128
    # split any group larger than MAXCALL into multiple calls
    calls = []  # list of (blk, chunk, n_idx) in processing order
    for b in range(NBLK):
        for c in range(NCHUNK):
            g = int(gpad[b, c])
            if g == 0:
                continue
            while g > 0:
                n = min(g, MAXCALL)
                calls.append((b, c, n))
                g -= n
    T = sum(n for _, _, n in calls) // 128  # tiles per core
    CW = sum(n // 16 for _, _, n in calls)

    # segment the call list at block boundaries so per-segment metadata
    # fits in SBUF; record (call_lo, call_hi, tile_lo, wcol_lo) per segment
    segs = []
    lo = 0
    tiles_acc = 0
    t_lo = 0
    w_lo = 0
    wcols_acc = 0
    for i, (b, c, n) in enumerate(calls):
        tiles_acc += n // 128
        wcols_acc += n // 16
        last_of_blk = (i + 1 == len(calls)) or calls[i + 1][0] != b
        if last_of_blk and (tiles_acc >= 384 or i + 1 == len(calls)):
            segs.append((lo, i + 1, t_lo, w_lo, tiles_acc, wcols_acc))
            lo = i + 1
            t_lo += tiles_acc
            w_lo += wcols_acc
            tiles_acc = 0
            wcols_acc = 0

    starts = np.zeros((NCORES, NBLK, NCHUNK), np.int64)
    flat = counts.reshape(NCORES, -1)
    np.cumsum(flat[:, :-1], axis=1, out=starts.reshape(NCORES, -1)[:, 1:])
    base = np.concatenate(([0], np.cumsum(np.bincount(core, minlength=NCORES))))

    per_core = []
    for cr in range(NCORES):
        cidx = np.zeros((128, CW), np.int16)
        ridx = np.zeros((128, CW), np.int16)
        roff = np.full((128, T), -1.0, np.float32)
        ptf = np.zeros((128, T), np.float32)
        wpos = 0
        tpos = 0
        for b in range(NBLK):
            for c in range(NCHUNK):
                gsz = int(gpad[b, c])
                if gsz == 0:
                    continue
                n_real = int(counts[cr, b, c])
                s = base[cr] + starts[cr, b, c]
                ccol = np.zeros(gsz, np.int64)
                crow = np.zeros(gsz, np.int64)   # local row in [0, CORE_ROWS)
                cro = np.full(gsz, -1.0, np.float32)
                cpt = np.zeros(gsz, np.float32)
                ccol[:n_real] = col_s[s:s + n_real] % CHUNK
                crow[:n_real] = row_s[s:s + n_real] % CORE_ROWS
                cro[:n_real] = (row_s[s:s + n_real] % CORE_ROWS) % R
                cpt[:n_real] = pt_s[s:s + n_real]
                # chop into calls of <= MAXCALL
                off = 0
                while off < gsz:
                    n = min(gsz - off, MAXCALL)
                    seg_c = ccol[off:off + n]
                    seg_r = crow[off:off + n]
                    cidx[:, wpos:wpos + n // 16] = np.tile(
                        seg_c.reshape(n // 16, 16).T.astype(np.int16), (8, 1))
                    ridx[:, wpos:wpos + n // 16] = np.tile(
                        seg_r.reshape(n // 16, 16).T.astype(np.int16), (8, 1))
                    nt = n // 128
                    roff[:, tpos:tpos + nt] = cro[off:off + n].reshape(nt, 128).T
                    ptf[:, tpos:tpos + nt] = cpt[off:off + n].reshape(nt, 128).T
                    wpos += n // 16
                    tpos += nt
                    off += n
        per_core.append(dict(cidx=cidx, ridx=ridx, roff=roff, ptf=ptf))
    return calls, T, CW, segs, per_core


def _build(calls, T, CW, segs, wvals):
    import concourse.mybir as mybir
    import concourse.tile as tile
    from concourse import bacc

    SEG_T = max(s[4] for s in segs)
    SEG_W = max(s[5] for s in segs)

    nc = bacc.Bacc(trn_type="TRN2", num_swdge_queues=4)
    kve = nc.dram_tensor("kve", [NPAD, KVE_B], mybir.dt.uint8, kind="ExternalInput")
    qe = nc.dram_tensor("qe", [CORE_ROWS, QE_W], mybir.dt.float32,
                        kind="ExternalInput")
    cidx = nc.dram_tensor("cidx", [128, CW], mybir.dt.int16, kind="ExternalInput")
    ridx = nc.dram_tensor("ridx", [128, CW], mybir.dt.int16, kind="ExternalInput")
    roff = nc.dram_tensor("roff", [128, T], mybir.dt.float32, kind="ExternalInput")
    ptf = nc.dram_tensor("ptf", [128, T], mybir.dt.float32, kind="ExternalInput")
    iota = nc.dram_tensor("iota", [128, R], mybir.dt.float32, kind="ExternalInput")
    raw = nc.dram_tensor("raw", [NBLK * 128, 257], mybir.dt.float32,
                         kind="ExternalOutput")

    with tile.TileContext(nc) as tc:
        with tc.tile_pool(name="const", bufs=1) as cpool, \
             tc.tile_pool(name="meta", bufs=2) as meta, \
             tc.tile_pool(name="gpool", bufs=4) as gpool, \
             tc.tile_pool(name="work", bufs=2) as work, \
             tc.tile_pool(name="psum", bufs=2, space="PSUM") as pp:
            iota_t = cpool.tile([128, R], mybir.dt.float32)
            nc.sync.dma_start(out=iota_t[:], in_=iota[:, :])

            for (clo, chi, t_lo, w_lo, nt_seg, nw_seg) in segs:
                cidx_t = meta.tile([128, SEG_W], mybir.dt.int16, tag="cidx")
                nc.sync.dma_start(out=cidx_t[:, :nw_seg],
                                  in_=cidx[:, w_lo:w_lo + nw_seg])
                ridx_t = meta.tile([128, SEG_W], mybir.dt.int16, tag="ridx")
                nc.sync.dma_start(out=ridx_t[:, :nw_seg],
                                  in_=ridx[:, w_lo:w_lo + nw_seg])
                roff_t = meta.tile([128, SEG_T], mybir.dt.float32, tag="roff")
                nc.sync.dma_start(out=roff_t[:, :nt_seg],
                                  in_=roff[:, t_lo:t_lo + nt_seg])
                ptf_t = meta.tile([128, SEG_T], mybir.dt.float32, tag="ptf")
                nc.sync.dma_start(out=ptf_t[:, :nt_seg],
                                  in_=ptf[:, t_lo:t_lo + nt_seg])

                # e1 = exp(path_emb_w)[path_type] via compare-mult ops
                e1_t = meta.tile([128, SEG_T], mybir.dt.float32, tag="e1")
                tmp_t = meta.tile([128, SEG_T], mybir.dt.float32, tag="tmp")
                acc_t = meta.tile([128, SEG_T], mybir.dt.float32, tag="acc")
                nc.vector.tensor_scalar(
                    out=e1_t[:, :nt_seg], in0=ptf_t[:, :nt_seg], scalar1=0.0,
                    scalar2=float(wvals[0]),
                    op0=mybir.AluOpType.is_equal, op1=mybir.AluOpType.mult)
                for p in range(1, P6):
                    nc.vector.tensor_scalar(
                        out=tmp_t[:, :nt_seg], in0=ptf_t[:, :nt_seg],
                        scalar1=float(p), scalar2=float(wvals[p]),
                        op0=mybir.AluOpType.is_equal, op1=mybir.AluOpType.mult)
                    src = e1_t if p % 2 == 1 else acc_t
                    dst = acc_t if p % 2 == 1 else e1_t
                    nc.vector.tensor_tensor(
                        out=dst[:, :nt_seg], in0=src[:, :nt_seg],
                        in1=tmp_t[:, :nt_seg], op=mybir.AluOpType.add)
                e1f = acc_t if (P6 - 1) % 2 == 1 else e1_t

                wpos = 0
                tpos = 0
                ps = None
                for ci in range(clo, chi):
                    b, c, n = calls[ci]
                    nt = n // 128
                    kg = gpool.tile([128, (MAXCALL // 128) * KVE_B],
                                    mybir.dt.uint8, tag="kg")
                    nc.gpsimd.dma_gather(
                        out_ap=kg[:, :nt * KVE_B].rearrange(
                            "p (n d) -> p n d", d=KVE_B),
                        in_ap=kve[c * CHUNK:(c + 1) * CHUNK, :],
                        idxs_ap=cidx_t[:, wpos:wpos + n // 16],
                        num_idxs=n, num_idxs_reg=n, elem_size=KVE_B,
                        queue_num=ci % 4)
                    qg = gpool.tile([128, (MAXCALL // 128) * QE_W],
                                    mybir.dt.float32, tag="qg")
                    nc.gpsimd.dma_gather(
                        out_ap=qg[:, :nt * QE_W].rearrange(
                            "p (n d) -> p n d", d=QE_W),
                        in_ap=qe[:],
                        idxs_ap=ridx_t[:, wpos:wpos + n // 16],
                        num_idxs=n, num_idxs_reg=n, elem_size=QE_W,
                        queue_num=(ci + 2) % 4)
                    wpos += n // 16

                    kf = kg[:].bitcast(mybir.dt.float32)
                    kb = kg[:].bitcast(mybir.dt.bfloat16)
                    prod = work.tile([128, (MAXCALL // 128) * 144],
                                     mybir.dt.float32, tag="prod")
                    qv = qg[:].rearrange("p (n d) -> p n d", d=QE_W)
                    kv = kf.rearrange("p (n d) -> p n d", d=KVE_B // 4)
                    nc.vector.tensor_tensor(
                        out=prod[:, :nt * 144].rearrange(
                            "p (n d) -> p n d", d=144),
                        in0=qv[:, :nt, 0:144], in1=kv[:, :nt, 0:144],
                        op=mybir.AluOpType.mult)
                    s0 = work.tile([128, MAXCALL // 128], mybir.dt.float32,
                                   tag="s0")
                    nc.vector.tensor_reduce(
                        out=s0[:, :nt],
                        in_=prod[:, :nt * 144].rearrange(
                            "p (n d) -> p n d", d=144),
                        axis=mybir.AxisListType.X, op=mybir.AluOpType.add)
                    e0 = work.tile([128, MAXCALL // 128], mybir.dt.float32,
                                   tag="e0")
                    nc.scalar.activation(
                        out=e0[:, :nt], in_=s0[:, :nt],
                        func=mybir.ActivationFunctionType.Exp)

                    A = work.tile([128, (MAXCALL // 128) * 2 * R],
                                  mybir.dt.bfloat16, tag="A")
                    oh = work.tile([128, (MAXCALL // 128) * R],
                                   mybir.dt.float32, tag="oh")
                    Av = A[:].rearrange("p (n d) -> p n d", d=2 * R)
                    ohv = oh[:].rearrange("p (n d) -> p n d", d=R)
                    nc.vector.tensor_tensor(
                        out=ohv[:, :nt, :],
                        in0=iota_t[:].rearrange("p (o d) -> p o d", o=1)
                            .to_broadcast([128, nt, R]),
                        in1=roff_t[:, tpos:tpos + nt].rearrange(
                            "p (n o) -> p n o", o=1).to_broadcast([128, nt, R]),
                        op=mybir.AluOpType.is_equal)
                    nc.vector.tensor_tensor(
                        out=Av[:, :nt, 0:R], in0=ohv[:, :nt, :],
                        in1=e0[:, :nt].rearrange("p (n o) -> p n o", o=1)
                            .to_broadcast([128, nt, R]),
                        op=mybir.AluOpType.mult)
                    nc.vector.tensor_tensor(
                        out=Av[:, :nt, R:2 * R], in0=ohv[:, :nt, :],
                        in1=e1f[:, tpos:tpos + nt].rearrange(
                            "p (n o) -> p n o", o=1).to_broadcast([128, nt, R]),
                        op=mybir.AluOpType.mult)

                    first = ps is None
                    if first:
                        ps = pp.tile([128, 257], mybir.dt.float32, tag="ps")
                    last_call_of_blk = (ci + 1 == len(calls)) or \
                        calls[ci + 1][0] != b
                    for j in range(nt):
                        rhs = kb[:, j * (KVE_B // 2) + 288:
                                 j * (KVE_B // 2) + 288 + 257]
                        nc.tensor.matmul(
                            out=ps[:], lhsT=A[:, j * 2 * R:(j + 1) * 2 * R],
                            rhs=rhs,
                            start=(first and j == 0),
                            stop=(last_call_of_blk and j == nt - 1))
                    tpos += nt

                    if last_call_of_blk:
                        ev = work.tile([128, 257], mybir.dt.float32, tag="ev")
                        nc.scalar.copy(out=ev[:], in_=ps[:])
                        nc.sync.dma_start(
                            out=raw[b * 128:(b + 1) * 128, :], in_=ev[:])
                        ps = None
    nc.finalize()
    return nc


def kernel(q, k, v, eigs, lambda0, path_emb_w, indices, path_type):
    _install_axon_hooks()
    q = np.asarray(q, np.float32)
    k = np.asarray(k, np.float32)
    v = np.asarray(v, np.float32)
    eigs = np.asarray(eigs, np.float32)
    lambda0 = np.asarray(lambda0, np.float32)
    path_emb_w = np.asarray(path_emb_w, np.float32)
    indices = np.asarray(indices, np.int32)
    path_type = np.asarray(path_type, np.int32)

    ew = float(np.exp(lambda0[0]))
    wvals = np.exp(path_emb_w[:, 0]).astype(np.float64)

    calls, T, CW, segs, per_core = _prep(indices, path_type)

    # kve table: [k f32 | eigs f32 | vhi bf16 | vlo bf16 | one bf16 | pad]
    kve = np.zeros((NPAD, KVE_B), np.uint8)
    kq = np.zeros((NPAD, 144), np.float32)
    kq[:N, :H] = k
    kq[:N, H:] = eigs
    kve[:, 0:576] = kq.view(np.uint8).reshape(NPAD, 576)
    from ml_dtypes import bfloat16
    vhi = v.astype(bfloat16)
    vlo = (v - vhi.astype(np.float32)).astype(bfloat16)
    kve[:N, 576:832] = vhi.view(np.uint8)
    kve[:N, 832:1088] = vlo.view(np.uint8)
    kve[:, 1088:1090] = np.tile(
        np.array([1.0], bfloat16).view(np.uint8), (NPAD, 1))

    qefull = np.zeros((NPAD, QE_W), np.float32)
    qefull[:N, :H] = q * (1.0 / np.sqrt(np.float32(H)))
    qefull[:N, H:144] = eigs * ew

    iota = np.tile(np.arange(R, dtype=np.float32), (128, 1))

    nc = _build(calls, T, CW, segs, wvals)

    in_maps = []
    for cr in range(NCORES):
        pc = per_core[cr]
        in_maps.append({
            "kve": kve,
            "qe": qefull[cr * CORE_ROWS:(cr + 1) * CORE_ROWS],
            "cidx": pc["cidx"], "ridx": pc["ridx"],
            "roff": pc["roff"], "ptf": pc["ptf"],
            "iota": iota,
        })

    from concourse.bass_utils import run_bass_kernel_spmd
    want_trace = bool(os.environ.get("KERNEL_TRACE"))
    res = run_bass_kernel_spmd(nc, in_maps, core_ids=list(range(NCORES)),
                               trace=want_trace)
    global LAST_EXEC_NS
    LAST_EXEC_NS = res.exec_time_ns

    out = np.zeros((NPAD, H), np.float32)
    for cr in range(NCORES):
        rawb = res.results[cr]["raw"].reshape(NBLK, 128, 257)
        p0 = rawb[:, 0:64, 0:128] + rawb[:, 0:64, 128:256]
        p1 = rawb[:, 64:128, 0:128] + rawb[:, 64:128, 128:256]
        d0 = rawb[:, 0:64, 256]
        d1 = rawb[:, 64:128, 256]
        d0 = np.where(d0 > 0, d0, 1.0)
        d1 = np.where(d1 > 0, d1, 1.0)
        blkout = 0.5 * (p0 / d0[..., None] + p1 / d1[..., None])
        out[cr * CORE_ROWS:(cr + 1) * CORE_ROWS] = blkout.reshape(CORE_ROWS, H)
    return out[:N]


if __name__ == "__main__":
    # small smoke test with synthetic inputs
    rng = np.random.default_rng(0)
    Et = int(os.environ.get("ET", "200000"))
    idx = rng.integers(0, N, size=(2, Et)).astype(np.int32)
    pt = rng.integers(0, P6, size=(Et,)).astype(np.int32)
    qq = rng.standard_normal((N, H), dtype=np.float32)
    kk = rng.standard_normal((N, H), dtype=np.float32)
    vv = rng.standard_normal((N, H), dtype=np.float32)
    ee = rng.standard_normal((N, ED), dtype=np.float32)
    l0 = np.zeros(1, np.float32)
    pw = rng.standard_normal((P6, 1), dtype=np.float32)

    out = kernel(qq, kk, vv, ee, l0, pw, idx, pt)

    # numpy reference
    row, col = idx[0], idx[1]
    x = (qq[row] * kk[col]).sum(-1) / np.sqrt(H) + np.exp(l0[0]) * (
        ee[row] * ee[col]).sum(-1)
    s1 = pw[pt, 0]
    exp0 = np.exp(x - x.max())
    d0 = np.zeros(N); np.add.at(d0, row, exp0)
    exp1 = np.exp(s1)
    d1 = np.zeros(N); np.add.at(d1, row, exp1)
    a = 0.5 * (exp0 / d0[row] + exp1 / d1[row])
    ref = np.zeros((N, H), np.float32)
    np.add.at(ref, row, a[:, None] * vv[col])
    num = np.linalg.norm(out - ref)
    den = np.linalg.norm(ref)
    print("rel err:", num / den)



# revision 14
# speedup vs baseline: 1.4137x; 1.4137x over previous
"""Trainium2 Bass kernel for GNN edge-softmax attention message passing.

Strategy (v1):
  - sort edges by destination row; 8 cores own contiguous 12544-row slices
  - 256-row superblocks; per superblock, edges grouped by col-chunk (4
    chunks of 25088 cols so dma_gather's int16 indices reach the whole
    node table); one gather call per (superblock, chunk) group
  - per edge, TWO 512B gathers sharing one index stream (both indexed by
    col): a TRANSPOSED gather of [k bf16 | eigs fp16] giving [d, e]
    layout, and a plain gather of [v bf16 | one] giving [e, d] layout
  - scores on the TensorEngine: S[e, r] = k[col_e].q[row_r]/sqrt(H)
    + eigs[col_e].eigs[row_r]*exp(l0) for the 64 rows of the edge's
    sub-block, via 2 matmuls against a per-superblock qT block loaded
    with straight DMA (no per-edge q gather)
  - A0 = exp((S+BIG)*onehot - BIG) on the Scalar engine selects each
    edge's own row AND produces the weighted one-hot matrix directly;
    A1 = onehot * exp(path_emb_w[path_type])
  - one matmul per 128-edge tile accumulates [sum e*v | sum e] for both
    channels into a per-64-row-sub-block PSUM tile [128, 129]
  - per-block raw [128, 129] results go back to HBM; the final
    0.5*(P0/d0 + P1/d1) combine runs on host during unsharding
"""

import os
import sys
import types

import numpy as np

N = 100000
E = 3200000
H = 128
ED = 16
P6 = 6
NCORES = 8
R = 64                  # rows per psum sub-block
SB = 256                # rows per superblock (4 sub-blocks)
SUBS = SB // R
CORE_ROWS = 12544       # 49 superblocks of 256 rows
NPAD = CORE_ROWS * NCORES
NSB = CORE_ROWS // SB   # 49
NBLK = CORE_ROWS // R   # 196
NCHUNK = 4
CHUNK = NPAD // NCHUNK  # 25088
KT_W = 256              # bf16 elems per kt row (512B): [k 128 | eigs-f16 16 | pad]
V_W = 256               # bf16 elems per v row (512B): [v 128 | one | pad]
MAXCALL = 896   # >896 idx breaks the transposed gather's single-packet DGE
BIG = 120.0  # exp(-BIG) underflows f32 to exactly 0 -> masked lanes vanish
SEG_SBS = 8             # superblocks per metadata segment
BATCH = 8               # score-matmul pairs batched per PSUM bank
LAST_EXEC_NS = None


def _install_axon_hooks():
    if "antenv.axon_hooks" in sys.modules:
        return
    mod = types.ModuleType("antenv.axon_hooks")
    _hook = [None]
    mod.set_axon_ntff_profile_hook = lambda h: _hook.__setitem__(0, h)
    mod.get_axon_ntff_profile_hook = lambda: _hook[0]
    sys.modules["antenv.axon_hooks"] = mod
    try:
        import antenv
        antenv.axon_hooks = mod
    except ImportError:
        pass
    try:
        from trn_agent_boot.trn_boot import _ntff_profile_via_ctypes
        h = _ntff_profile_via_ctypes("/opt/axon/libaxon_pjrt.so")
        if h is not None:
            mod.set_axon_ntff_profile_hook(h)
    except Exception:
        pass


def _prep(indices, path_type):
    """Sort/pad edges; build the shared call/tile/pair plan + per-core data."""
    row = indices[0].astype(np.int64)
    col = indices[1].astype(np.int64)
    core = row // CORE_ROWS
    sb = (row % CORE_ROWS) // SB
    loc = row % SB
    chunk = col // CHUNK
    gkey = (core * NSB + sb) * NCHUNK + chunk
    order = np.argsort(gkey * SB + loc, kind="stable")
    loc_s = loc[order]
    cidx_s = (col[order] % CHUNK).astype(np.int16)
    pt_s = path_type[order].astype(np.float32)
    gkey_s = gkey[order]

    ngroups = NCORES * NSB * NCHUNK
    counts = np.bincount(gkey_s, minlength=ngroups).reshape(NCORES, NSB, NCHUNK)
    gmax = counts.max(axis=0)
    gpad = ((gmax + 127) // 128) * 128      # [NSB, NCHUNK]

    # group starts within each core's sorted slab
    starts = np.zeros((NCORES, NSB * NCHUNK), np.int64)
    flat = counts.reshape(NCORES, -1)
    np.cumsum(flat[:, :-1], axis=1, out=starts[:, 1:])
    base = np.concatenate(([0], np.cumsum(np.bincount(core, minlength=NCORES))))

    # padded flat layout (same for all cores)
    gpad_flat = gpad.reshape(-1)
    pstart = np.zeros(NSB * NCHUNK, np.int64)
    np.cumsum(gpad_flat[:-1], out=pstart[1:])
    EP = int(gpad_flat.sum())

    # per-core padded per-edge arrays
    locp = np.full((NCORES, EP), -999.0, np.float32)
    cidxp = np.zeros((NCORES, EP), np.int16)
    ptp = np.zeros((NCORES, EP), np.float32)
    for cr in range(NCORES):
        for g in range(NSB * NCHUNK):
            n = int(flat[cr, g])
            if n == 0:
                continue
            s0 = base[cr] + starts[cr, g]
            d0 = pstart[g]
            locp[cr, d0:d0 + n] = loc_s[s0:s0 + n]
            cidxp[cr, d0:d0 + n] = cidx_s[s0:s0 + n]
            ptp[cr, d0:d0 + n] = pt_s[s0:s0 + n]

    # calls: (sb, chunk, n, padded-offset); split at MAXCALL
    calls = []
    for s_ in range(NSB):
        for c in range(NCHUNK):
            g = int(gpad[s_, c])
            off = int(pstart[s_ * NCHUNK + c])
            while g > 0:
                n = min(g, MAXCALL)
                calls.append((s_, c, n, off))
                off += n
                g -= n

    # pairs: per call/tile, union of touched sub-blocks across cores
    # pair record: (call_idx, tile_in_call, s)
    pairs = []
    call_pairs = []   # per call: list of pair dicts
    for ci, (s_, c, n, off) in enumerate(calls):
        plist = []
        for jj in range(n // 128):
            tl = locp[:, off + jj * 128: off + (jj + 1) * 128]
            subs = np.unique(tl[tl >= 0] // 64).astype(np.int64)
            for s in subs:
                plist.append(dict(j=jj, s=int(s), idx=len(pairs)))
                pairs.append((ci, jj, int(s)))
        call_pairs.append(plist)
    NPAIRS = len(pairs)

    # start/stop flags per (sb, s) psum accumulation group
    groups = {}
    for ci, plist in enumerate(call_pairs):
        s_ = calls[ci][0]
        for pr in plist:
            groups.setdefault((s_, pr["s"]), []).append(pr)
    for (s_, s), lst in groups.items():
        for pr in lst:
            pr["start"] = False
            pr["stop"] = False
        lst[0]["start"] = True
        lst[-1]["stop"] = True
    started = set(groups.keys())

    # per-core metadata arrays
    CW = sum(n // 16 for _, _, n, _ in calls)
    roffp = np.full((NCORES, 128, NPAIRS), -999.0, np.float32)
    ptfp = np.zeros((NCORES, 128, NPAIRS), np.float32)
    cidx = np.zeros((NCORES, 128, CW), np.int16)
    wpos = 0
    for ci, (s_, c, n, off) in enumerate(calls):
        for cr in range(NCORES):
            seg = cidxp[cr, off:off + n]
            cidx[cr, :, wpos:wpos + n // 16] = np.tile(
                seg.reshape(n // 16, 16).T, (8, 1))
        for pr in call_pairs[ci]:
            jj, s = pr["j"], pr["s"]
            tl = locp[:, off + jj * 128: off + (jj + 1) * 128]
            tp = ptp[:, off + jj * 128: off + (jj + 1) * 128]
            rr = np.where(tl >= 0, tl - 64.0 * s, -999.0)
            roffp[:, :, pr["idx"]] = rr
            ptfp[:, :, pr["idx"]] = tp
        wpos += n // 16

    # segments: SEG_SBS superblocks each; record call ranges + offsets
    segs = []   # (call_lo, call_hi, w_lo, w_n, p_lo, p_n)
    ci = 0
    w_lo = p_lo = 0
    while ci < len(calls):
        sb_lo = calls[ci][0]
        cj = ci
        w_n = p_n = 0
        while cj < len(calls) and calls[cj][0] < sb_lo + SEG_SBS:
            w_n += calls[cj][2] // 16
            p_n += len(call_pairs[cj])
            cj += 1
        segs.append((ci, cj, w_lo, w_n, p_lo, p_n))
        w_lo += w_n
        p_lo += p_n
        ci = cj

    MAXN = max(n for _, _, n, _ in calls)
    NPC = max((len(pl) for pl in call_pairs), default=1)
    plan = dict(calls=calls, call_pairs=call_pairs, segs=segs,
                CW=CW, NPAIRS=NPAIRS, MAXN=MAXN, NPC=NPC,
                started=started)
    per_core = [dict(cidx=cidx[cr], roffp=roffp[cr], ptfp=ptfp[cr])
                for cr in range(NCORES)]
    return plan, per_core


def _build(plan, wvals):
    import concourse.mybir as mybir
    import concourse.tile as tile
    from concourse import bacc

    calls = plan["calls"]
    call_pairs = plan["call_pairs"]
    segs = plan["segs"]
    CW, NPAIRS, MAXN, NPC = plan["CW"], plan["NPAIRS"], plan["MAXN"], plan["NPC"]
    SEG_W = max(s[3] for s in segs)
    SEG_P = max(s[5] for s in segs)

    nc = bacc.Bacc(trn_type="TRN2", num_swdge_queues=4)
    kt = nc.dram_tensor("kt", [NPAD, KT_W], mybir.dt.bfloat16, kind="ExternalInput")
    vt = nc.dram_tensor("vt", [NPAD, V_W], mybir.dt.bfloat16, kind="ExternalInput")
    qt1 = nc.dram_tensor("qt1", [128, CORE_ROWS], mybir.dt.bfloat16,
                         kind="ExternalInput")
    qt2 = nc.dram_tensor("qt2", [16, CORE_ROWS], mybir.dt.float16,
                         kind="ExternalInput")
    cidx = nc.dram_tensor("cidx", [128, CW], mybir.dt.int16, kind="ExternalInput")
    roffp = nc.dram_tensor("roffp", [128, NPAIRS], mybir.dt.float32,
                           kind="ExternalInput")
    ptfp = nc.dram_tensor("ptfp", [128, NPAIRS], mybir.dt.float32,
                          kind="ExternalInput")
    iota = nc.dram_tensor("iota", [128, R], mybir.dt.float32, kind="ExternalInput")
    raw = nc.dram_tensor("raw", [NBLK * 128, 129], mybir.dt.float32,
                         kind="ExternalOutput")

    f32 = mybir.dt.float32
    bf16 = mybir.dt.bfloat16
    fp16 = mybir.dt.float16
    Alu = mybir.AluOpType

    with tile.TileContext(nc) as tc:
        with tc.tile_pool(name="const", bufs=1) as cpool, \
             tc.tile_pool(name="qpool", bufs=2) as qpool, \
             tc.tile_pool(name="meta", bufs=2) as meta, \
             tc.tile_pool(name="gpool", bufs=4) as gpool, \
             tc.tile_pool(name="work", bufs=3) as work, \
             tc.tile_pool(name="evp", bufs=4) as evp, \
             tc.tile_pool(name="sfp", bufs=2, space="PSUM") as sfp, \
             tc.tile_pool(name="opp", bufs=1, space="PSUM") as opp:
            iota_t = cpool.tile([128, R], f32)
            nc.sync.dma_start(out=iota_t[:], in_=iota[:, :])
            nbig_t = cpool.tile([128, 1], f32)
            nc.vector.memset(nbig_t[:], -BIG)

            qnum = 0
            for (clo, chi, w_lo, w_n, p_lo, p_n) in segs:
                cidx_t = meta.tile([128, SEG_W], mybir.dt.int16, tag="cidx")
                nc.sync.dma_start(out=cidx_t[:, :w_n],
                                  in_=cidx[:, w_lo:w_lo + w_n])
                rp_t = meta.tile([128, SEG_P], f32, tag="rp")
                nc.sync.dma_start(out=rp_t[:, :p_n],
                                  in_=roffp[:, p_lo:p_lo + p_n])
                pt_t = meta.tile([128, SEG_P], f32, tag="pt")
                nc.sync.dma_start(out=pt_t[:, :p_n],
                                  in_=ptfp[:, p_lo:p_lo + p_n])

                # e1 = exp(path_emb_w)[path_type] via compare-mult chain
                e1_t = meta.tile([128, SEG_P], f32, tag="e1")
                tmp_t = meta.tile([128, SEG_P], f32, tag="tmp")
                acc_t = meta.tile([128, SEG_P], f32, tag="acc")
                nc.vector.tensor_scalar(
                    out=e1_t[:, :p_n], in0=pt_t[:, :p_n], scalar1=0.0,
                    scalar2=float(wvals[0]),
                    op0=Alu.is_equal, op1=Alu.mult)
                for p in range(1, P6):
                    nc.vector.tensor_scalar(
                        out=tmp_t[:, :p_n], in0=pt_t[:, :p_n],
                        scalar1=float(p), scalar2=float(wvals[p]),
                        op0=Alu.is_equal, op1=Alu.mult)
                    src = e1_t if p % 2 == 1 else acc_t
                    dst = acc_t if p % 2 == 1 else e1_t
                    nc.vector.tensor_tensor(
                        out=dst[:, :p_n], in0=src[:, :p_n],
                        in1=tmp_t[:, :p_n], op=Alu.add)
                e1f = acc_t if (P6 - 1) % 2 == 1 else e1_t

                wpos = 0
                ppos = 0
                cur_sb = -1
                o_t = {}

                def flush_sb(sb_idx):
                    for s in range(SUBS):
                        ot = o_t.pop(s, None)
                        if ot is None:
                            continue
                        ev = evp.tile([128, 129], f32, tag="ev")
                        nc.scalar.copy(out=ev[:], in_=ot[:, 0:129])
                        b = sb_idx * SUBS + s
                        nc.sync.dma_start(
                            out=raw[b * 128:(b + 1) * 128, :], in_=ev[:])

                for ci in range(clo, chi):
                    sb_i, c, n, off = calls[ci]
                    plist = call_pairs[ci]
                    npc = len(plist)
                    nt = n // 128
                    if sb_i != cur_sb:
                        if cur_sb >= 0:
                            flush_sb(cur_sb)
                        cur_sb = sb_i
                        q1_t = qpool.tile([128, SB], bf16, tag="q1")
                        nc.sync.dma_start(
                            out=q1_t[:],
                            in_=qt1[:, sb_i * SB:(sb_i + 1) * SB])
                        q2_t = qpool.tile([16, SB], fp16, tag="q2")
                        nc.sync.dma_start(
                            out=q2_t[:],
                            in_=qt2[:, sb_i * SB:(sb_i + 1) * SB])

                    ktile = gpool.tile([128, 2 * MAXN], bf16, tag="kt")
                    ktv = ktile[:, :2 * n].rearrange("p (c n) -> p c n", c=2)
                    nc.gpsimd.dma_gather(
                        out_ap=ktv,
                        in_ap=kt[c * CHUNK:(c + 1) * CHUNK, :],
                        idxs_ap=cidx_t[:, wpos:wpos + n // 16],
                        num_idxs=n, num_idxs_reg=n, elem_size=KT_W,
                        transpose=True, queue_num=qnum % 4)
                    qnum += 1
                    vtile = gpool.tile([128, (MAXN // 128) * V_W], bf16,
                                       tag="vt")
                    vv = vtile[:, :nt * V_W].rearrange("p (n d) -> p n d",
                                                       d=V_W)
                    nc.gpsimd.dma_gather(
                        out_ap=vv,
                        in_ap=vt[c * CHUNK:(c + 1) * CHUNK, :],
                        idxs_ap=cidx_t[:, wpos:wpos + n // 16],
                        num_idxs=n, num_idxs_reg=n, elem_size=V_W,
                        queue_num=qnum % 4)
                    qnum += 1
                    wpos += n // 16
                    kt16 = ktile[:, :2 * n].bitcast(fp16).rearrange(
                        "p (c n) -> p c n", c=2)

                    # one-hot for every pair of the call, one DVE op
                    oh_t = work.tile([128, NPC * R], f32, tag="oh")
                    ohv = oh_t[:, :npc * R].rearrange("p (n d) -> p n d", d=R)
                    nc.vector.tensor_tensor(
                        out=ohv,
                        in0=rp_t[:, ppos:ppos + npc].rearrange(
                            "p (n o) -> p n o", o=1).to_broadcast(
                            [128, npc, R]),
                        in1=iota_t[:].rearrange("p (o d) -> p o d",
                                                o=1).to_broadcast(
                            [128, npc, R]),
                        op=Alu.is_equal)
                    A_t = work.tile([128, NPC * 2 * R], bf16, tag="A")
                    Av = A_t[:, :npc * 2 * R].rearrange("p (n d) -> p n d",
                                                        d=2 * R)
                    nc.vector.tensor_tensor(
                        out=Av[:, :, R:2 * R], in0=ohv,
                        in1=e1f[:, ppos:ppos + npc].rearrange(
                            "p (n o) -> p n o", o=1).to_broadcast(
                            [128, npc, R]),
                        op=Alu.mult)

                    for b0 in range(0, npc, BATCH):
                        nb = min(BATCH, npc - b0)
                        sf_t = sfp.tile([128, BATCH * R], f32, tag="sf")
                        for bi in range(nb):
                            pr = plist[b0 + bi]
                            jj, s = pr["j"], pr["s"]
                            out_sl = sf_t[:, bi * R:(bi + 1) * R]
                            nc.tensor.matmul(
                                out=out_sl,
                                lhsT=ktv[:, 0, jj * 128:(jj + 1) * 128],
                                rhs=q1_t[:, s * R:(s + 1) * R],
                                start=True, stop=False)
                            nc.tensor.matmul(
                                out=out_sl,
                                lhsT=kt16[0:16, 1, jj * 128:(jj + 1) * 128],
                                rhs=q2_t[:, s * R:(s + 1) * R],
                                start=False, stop=True)
                        sm_t = work.tile([128, BATCH * R], f32, tag="sm")
                        smv = sm_t[:, :nb * R].rearrange("p (n d) -> p n d",
                                                         d=R)
                        # sm = oh*BIG + S; exp(sm - BIG) = exp(S) where
                        # oh=1, else exp(S-BIG) <= 1.2e-37 ~ 0
                        nc.vector.scalar_tensor_tensor(
                            out=smv,
                            in0=ohv[:, b0:b0 + nb, :],
                            scalar=BIG,
                            in1=sf_t[:, :nb * R].rearrange(
                                "p (n d) -> p n d", d=R),
                            op0=Alu.mult, op1=Alu.add)
                        nc.scalar.activation(
                            out=Av[:, b0:b0 + nb, 0:R], in_=smv,
                            func=mybir.ActivationFunctionType.Exp,
                            bias=nbig_t[:, 0:1])
                        for bi in range(nb):
                            pr = plist[b0 + bi]
                            jj, s = pr["j"], pr["s"]
                            if s not in o_t:
                                o_t[s] = opp.tile([128, 512], f32,
                                                  tag=f"o{s}",
                                                  name=f"opsum{s}")
                            nc.tensor.matmul(
                                out=o_t[s][:, 0:129],
                                lhsT=Av[:, b0 + bi, :],
                                rhs=vv[:, jj, 0:129],
                                start=pr["start"], stop=pr["stop"])
                    ppos += npc
                if cur_sb >= 0:
                    flush_sb(cur_sb)
    nc.finalize()
    return nc


def kernel(q, k, v, eigs, lambda0, path_emb_w, indices, path_type):
    _install_axon_hooks()
    q = np.asarray(q, np.float32)
    k = np.asarray(k, np.float32)
    v = np.asarray(v, np.float32)
    eigs = np.asarray(eigs, np.float32)
    lambda0 = np.asarray(lambda0, np.float32)
    path_emb_w = np.asarray(path_emb_w, np.float32)
    indices = np.asarray(indices, np.int32)
    path_type = np.asarray(path_type, np.int32)

    from ml_dtypes import bfloat16

    ew = float(np.exp(lambda0[0]))
    wvals = np.exp(path_emb_w[:, 0]).astype(np.float64)

    plan, per_core = _prep(indices, path_type)

    # kt table: [k bf16 128 | eigs fp16 16 | pad] per node row (512B)
    kt_np = np.zeros((NPAD, KT_W), bfloat16)
    kt_np[:N, :H] = k.astype(bfloat16)
    kt_u8 = kt_np.view(np.uint8).reshape(NPAD, KT_W * 2)
    kt_u8[:N, 256:288] = eigs.astype(np.float16).view(np.uint8)

    # v table: [v bf16 128 | one | pad] (512B)
    vt_np = np.zeros((NPAD, V_W), bfloat16)
    vt_np[:N, :H] = v.astype(bfloat16)
    vt_np[:, H] = bfloat16(1.0)

    # transposed q tables
    qt1_np = np.zeros((128, NPAD), bfloat16)
    qt1_np[:, :N] = (q.T * (1.0 / np.sqrt(np.float32(H)))).astype(bfloat16)
    qt2_np = np.zeros((16, NPAD), np.float16)
    qt2_np[:, :N] = (eigs.T * ew).astype(np.float16)

    iota_np = np.tile(np.arange(R, dtype=np.float32), (128, 1))

    nc = _build(plan, wvals)

    in_maps = []
    for cr in range(NCORES):
        pc = per_core[cr]
        in_maps.append({
            "kt": kt_np,
            "vt": vt_np,
            "qt1": np.ascontiguousarray(
                qt1_np[:, cr * CORE_ROWS:(cr + 1) * CORE_ROWS]),
            "qt2": np.ascontiguousarray(
                qt2_np[:, cr * CORE_ROWS:(cr + 1) * CORE_ROWS]),
            "cidx": pc["cidx"], "roffp": pc["roffp"], "ptfp": pc["ptfp"],
            "iota": iota_np,
        })

    from concourse.bass_utils import run_bass_kernel_spmd
    want_trace = bool(os.environ.get("KERNEL_TRACE"))
    res = run_bass_kernel_spmd(nc, in_maps, core_ids=list(range(NCORES)),
                               trace=want_trace)
    global LAST_EXEC_NS
    LAST_EXEC_NS = res.exec_time_ns

    started = plan["started"]
    out = np.zeros((NPAD, H), np.float32)
    for cr in range(NCORES):
        rawb = res.results[cr]["raw"].reshape(NBLK, 128, 129)
        mask = np.zeros((NBLK, 1, 1), np.float32)
        for (sb_i, s) in started:
            mask[sb_i * SUBS + s] = 1.0
        rawb = rawb * mask
        p0 = rawb[:, 0:64, 0:128]
        p1 = rawb[:, 64:128, 0:128]
        d0 = rawb[:, 0:64, 128]
        d1 = rawb[:, 64:128, 128]
        d0 = np.where(d0 > 0, d0, 1.0)
        d1 = np.where(d1 > 0, d1, 1.0)
        blkout = 0.5 * (p0 / d0[..., None] + p1 / d1[..., None])
        out[cr * CORE_ROWS:(cr + 1) * CORE_ROWS] = blkout.reshape(CORE_ROWS, H)
    return out[:N]


if __name__ == "__main__":
    # small smoke test with synthetic inputs
    rng = np.random.default_rng(0)
    Et = int(os.environ.get("ET", "200000"))
    idx = rng.integers(0, N, size=(2, Et)).astype(np.int32)
    pt = rng.integers(0, P6, size=(Et,)).astype(np.int32)
    qq = rng.standard_normal((N, H), dtype=np.float32)
    kk = rng.standard_normal((N, H), dtype=np.float32)
    vv = rng.standard_normal((N, H), dtype=np.float32)
    ee = rng.standard_normal((N, ED), dtype=np.float32)
    l0 = np.zeros(1, np.float32)
    pw = rng.standard_normal((P6, 1), dtype=np.float32)

    out = kernel(qq, kk, vv, ee, l0, pw, idx, pt)

    # numpy reference
    row, col = idx[0], idx[1]
    x = (qq[row] * kk[col]).sum(-1) / np.sqrt(H) + np.exp(l0[0]) * (
        ee[row] * ee[col]).sum(-1)
    s1 = pw[pt, 0]
    exp0 = np.exp(x)
    d0 = np.zeros(N); np.add.at(d0, row, exp0)
    exp1 = np.exp(s1)
    d1 = np.zeros(N); np.add.at(d1, row, exp1)
    a = 0.5 * (exp0 / np.maximum(d0[row], 1e-30) + exp1 / d1[row])
    ref = np.zeros((N, H), np.float32)
    np.add.at(ref, row, a[:, None] * vv[col])
    num = np.linalg.norm(out - ref)
    den = np.linalg.norm(ref)
    print("rel err:", num / den)
